# revision 22
# baseline (speedup 1.0000x reference)
"""AttentionSubsample Trainium2 kernel.

Full (unsharded) inputs in, full output out. Data-parallel over batch:
32 batches -> 8 NeuronCores x 4 batches each. Weights/biases replicated.

Engine-balance design (cost-model 206.4us/core, vs 275.6us v1 baseline):
  - k-channel BN bias dropped entirely: softmax over n is invariant to
    per-q shifts and (k+bk)@q shifts every key n equally.
  - score bias added pre-exp on the PE as fp8(e4m3) DoubleRow identity
    matmuls (0.5 cyc/row): lhsT=(I,0)/(0,I) selects one chunk of an
    adjacent bias-chunk pair, so the bias stays resident in SBUF stored
    once (3.2KB/partition/head, loaded one time, no per-batch DMA).
  - v projection as fp8 hi/lo split (x = x8h + x8l, Wv = w8h + w8l) with
    three K=256 DoubleRow passes per psum tile, dropping the lo*lo term:
    ~2.7x fewer PE cycles than f32r at bf16-level accuracy. kT/q stay
    f32r: their quantization noise would amplify through exp by sqrt(d).
  - softmax sums: e-tiles accumulated on DVE (bf16 2x mode, in-place
    chain) + one ones-matmul per head instead of 10 PE ones-matmuls.
  - hswish on Pool/DVE: t = min(Relu(o+3+bv), 6) via Pool tensor_scalar
    ops, th = (o+bv)*t via Pool scalar_tensor_tensor; normalize mult on
    Pool; bv folds out of attn@v (softmax rows sum to 1).
  - psum->sbuf copies split across ACT/DVE (GPSIMD cannot touch PSUM on
    real hw); out-proj bias fused into the DVE psum->sbuf add.
  - software pipelining: batch b+1's kT/q/v projection psum tiles are
    emitted interleaved between batch b's attention score groups (one
    tile after every group, via generators), and the pair output
    projection interleaves with the following batch, keeping the PE fed
    through the shared psum-pool rotation.
  - PSUM: scores pool 3x[128,2,512] (chunk pairs at bank-aligned 512
    offsets, exp reads the [*, :320] pair in one ACT instr), po + psm
    1 bank each = 8 banks.
"""

import sys

if "/opt/trn_rl_repo" not in sys.path:
    sys.path.insert(0, "/opt/trn_rl_repo")

import ml_dtypes
import numpy as np

# --- problem constants (hardcoded, must match the grading reference) ---
B, N, C = 32, 1280, 256
H, KD, D = 8, 64, 128          # heads, key dim, value dim per head
NQ = 320                       # subsampled sequence length
OUT = 384
NCORES = 8
BPC = B // NCORES              # batches per core
EPS = 1e-5
NCH = N // 128                 # 10 n-chunks of 128
GRP = 2                        # scores psum group size (n-chunks per group)

# per-head engine tuning: bias add on PE (fp8 DoubleRow) vs DVE (exp-bias mult)
BIAS_PE = [True] * 8
# per-head: softmax sums via 10 PE ones-matmuls vs DVE accumulate + 1 matmul
SUMS_PE = [False] * 8

_PE_HEADS = [h for h in range(H) if BIAS_PE[h]]
_DVE_HEADS = [h for h in range(H) if not BIAS_PE[h]]
_PE_SLOT = {h: i for i, h in enumerate(_PE_HEADS)}
_DVE_SLOT = {h: i for i, h in enumerate(_DVE_HEADS)}

_SUB_IDX = np.concatenate([
    (np.arange(32)[::2][:, None] * 32 + np.arange(32)[::2][None, :]).reshape(-1),
    1024 + (np.arange(16)[::2][:, None] * 16 + np.arange(16)[::2][None, :]).reshape(-1),
])  # [320] subsample row gather


def _prep(inputs):
    """Host-side: fold BN into weights, reorder channels, shard over cores."""
    f32 = np.float32
    x = np.asarray(inputs["x"], f32)
    g_kv, b_kv = np.asarray(inputs["g_kv"], f32), np.asarray(inputs["b_kv"], f32)
    rm_kv, rv_kv = np.asarray(inputs["rm_kv"], f32), np.asarray(inputs["rv_kv"], f32)
    g_q, b_q = np.asarray(inputs["g_q"], f32), np.asarray(inputs["b_q"], f32)
    rm_q, rv_q = np.asarray(inputs["rm_q"], f32), np.asarray(inputs["rv_q"], f32)
    g_p, b_p = np.asarray(inputs["g_p"], f32), np.asarray(inputs["b_p"], f32)
    rm_p, rv_p = np.asarray(inputs["rm_p"], f32), np.asarray(inputs["rv_p"], f32)
    W_kv = np.asarray(inputs["W_kv"], f32)
    W_q = np.asarray(inputs["W_q"], f32)
    W_p = np.asarray(inputs["W_p"], f32)
    attn_bias = np.asarray(inputs["attn_bias"], f32)
    bias_idxs = np.asarray(inputs["bias_idxs"])

    s_kv = g_kv / np.sqrt(rv_kv + EPS)
    Wkv_f = W_kv * s_kv[:, None]
    bkv_f = b_kv - rm_kv * s_kv
    kidx = np.concatenate([np.arange(h * 192, h * 192 + KD) for h in range(H)])
    vidx = np.concatenate([np.arange(h * 192 + KD, (h + 1) * 192) for h in range(H)])
    wkt = np.ascontiguousarray(Wkv_f[kidx].T).reshape(2, 128, 512)     # [c,128][512 kch]
    wvt = np.ascontiguousarray(Wkv_f[vidx].T).reshape(2, 128, 1024)
    bvd = np.ascontiguousarray(bkv_f[vidx].reshape(8, 128).T)          # [128, H]

    scale = KD ** -0.5
    s_q = g_q / np.sqrt(rv_q + EPS)
    wqt = np.ascontiguousarray((W_q * (s_q * scale)[:, None]).T).reshape(2, 128, 512)
    bq = np.ascontiguousarray(((b_q - rm_q * s_q) * scale).reshape(4, 128).T)

    f8_ = ml_dtypes.float8_e4m3
    wk8h = wkt.astype(f8_)
    wk8l = (wkt - wk8h.astype(np.float32)).astype(f8_)
    wq8h = wqt.astype(f8_)
    wq8l = (wqt - wq8h.astype(np.float32)).astype(f8_)

    s_p = g_p / np.sqrt(rv_p + EPS)
    wpt = np.ascontiguousarray((W_p * s_p[:, None]).T / 6.0).reshape(
        8, 128, OUT).astype(ml_dtypes.bfloat16)
    bps = np.ascontiguousarray(np.broadcast_to(b_p - rm_p * s_p, (128, OUT))).astype(np.float32)

    biasT = attn_bias[:, bias_idxs].transpose(0, 2, 1)                 # [H, N, NQ]
    bias_cpq = biasT.reshape(H, NCH, 128, NQ).transpose(0, 2, 1, 3)    # [H,128,NCH,NQ]
    f8 = ml_dtypes.float8_e4m3
    # bias fp8, stored once per head; the DoubleRow identity pair (I,0)/(0,I)
    # selects one chunk of an adjacent pair per instruction
    bt8 = np.ascontiguousarray(bias_cpq).astype(f8)                    # [H,128,NCH,NQ]

    identp = np.zeros((128, 2, 2, 128), f8)
    identp[np.arange(128), 0, 0, np.arange(128)] = 1.0
    identp[np.arange(128), 1, 1, np.arange(128)] = 1.0

    wv8h = wvt.astype(f8)
    wv8l = (wvt - wv8h.astype(np.float32)).astype(f8)

    xs = x[:, _SUB_IDX, :]                                             # [B, NQ, C]
    in_maps = []
    for i in range(NCORES):
        sl = slice(i * BPC, (i + 1) * BPC)
        xt = np.ascontiguousarray(x[sl].transpose(0, 2, 1)).reshape(BPC, 2, 128, N)
        x8h = xt.astype(f8)
        x8l = (xt - x8h.astype(np.float32)).astype(f8)
        xst = np.ascontiguousarray(xs[sl].transpose(0, 2, 1)).reshape(BPC, 2, 128, NQ)
        xs8h = xst.astype(f8)
        xs8l = (xst - xs8h.astype(np.float32)).astype(f8)
        in_maps.append({
            "x8h": x8h, "x8l": x8l, "xs8h": xs8h, "xs8l": xs8l,
            "wv8h": wv8h, "wv8l": wv8l,
            "wk8h": wk8h, "wk8l": wk8l, "wq8h": wq8h, "wq8l": wq8l,
            "wpt": wpt,
            "bq": bq, "bv": bvd, "bv3": bvd + 3.0, "bps": bps,
            "bt8": bt8,
            "ones": np.ones((128, 128), ml_dtypes.bfloat16),
            "identp": identp,
        })
    return in_maps


def _body(tc, a, out_ap):
    import concourse.bass as bass  # noqa: F401
    import concourse.mybir as mybir
    from contextlib import ExitStack

    nc = tc.nc
    f32 = mybir.dt.float32
    f32r = mybir.dt.float32r
    bf16 = mybir.dt.bfloat16
    f8e4 = mybir.dt.float8e4
    AF = mybir.ActivationFunctionType
    ALU = mybir.AluOpType
    PM = mybir.MatmulPerfMode

    with ExitStack() as ctx:
        ctx.enter_context(
            nc.allow_low_precision(reason="bf16 o-side + fp8 bias matmuls are deliberate; verified vs fp32 reference")
        )
        singles = ctx.enter_context(tc.tile_pool(name="singles", bufs=1))
        # DMA order matters at startup: first-needed weights first (wk8 ->
        # kT projection of batch 0), small attention-phase tiles later.
        wk8h = singles.tile([128, 2, 512], f8e4)
        nc.sync.dma_start(wk8h, a["wk8h"].rearrange("c p j -> p c j"))
        wk8l = singles.tile([128, 2, 512], f8e4)
        nc.sync.dma_start(wk8l, a["wk8l"].rearrange("c p j -> p c j"))
        wq8h = singles.tile([128, 2, 512], f8e4)
        wq8l = singles.tile([128, 2, 512], f8e4)
        bqs = singles.tile([128, 4], f32)
        wv8h = singles.tile([128, 2, 1024], f8e4)
        wv8l = singles.tile([128, 2, 1024], f8e4)
        wp = singles.tile([128, 8, OUT], bf16)
        bvs = singles.tile([128, H], f32)
        bvs3 = singles.tile([128, H], f32)
        ones = singles.tile([128, 128], bf16)
        identp = singles.tile([128, 2, 2, 128], f8e4)
        bps = singles.tile([128, OUT], f32)
        bt8s = [singles.tile([128, NCH, NQ], f8e4, name=f"bt8h{h}")
                for h in range(H)]

        # bufs=2: batch b+1's input DMA must not wait on batch b's projection
        # reads -- a bufs=1 ring would stall the DMA on the SP SEQ, blocking
        # every later-issued DMA (bt8, wp) behind it
        x8_p = ctx.enter_context(tc.tile_pool(name="x8", bufs=2))
        xs8_p = ctx.enter_context(tc.tile_pool(name="xs8", bufs=2))
        kt_p = ctx.enter_context(tc.tile_pool(name="kt", bufs=2))
        v_p = ctx.enter_context(tc.tile_pool(name="v", bufs=2))
        qt_p = ctx.enter_context(tc.tile_pool(name="qt", bufs=3))
        e_p = ctx.enter_context(tc.tile_pool(name="e", bufs=10))
        esum_p = ctx.enter_context(tc.tile_pool(name="esum", bufs=3))
        rc_p = ctx.enter_context(tc.tile_pool(name="rc", bufs=3))
        oh_p = ctx.enter_context(tc.tile_pool(name="oh", bufs=3))
        t1_p = ctx.enter_context(tc.tile_pool(name="t1", bufs=3))
        hs_p = ctx.enter_context(tc.tile_pool(name="hs", bufs=2))
        ob_p = ctx.enter_context(tc.tile_pool(name="ob", bufs=4))
        ps_sg = ctx.enter_context(tc.tile_pool(name="ps_sg", bufs=3, space="PSUM"))
        ps_o = ctx.enter_context(tc.tile_pool(name="ps_o", bufs=1, space="PSUM"))
        ps_sum = ctx.enter_context(tc.tile_pool(name="ps_sum", bufs=1, space="PSUM"))

        _wt_n = [0]

        def sg_tile():
            _wt_n[0] += 1
            return ps_sg.tile([128, GRP, 512], f32, tag="sg", name=f"sg{_wt_n[0]}")

        out_flat = out_ap.rearrange("b q o -> (b q) o")

        def dma_x(b, first=False, stagger=False):
            """Issue input DMAs for batch b; returns (x8h, x8l, xs8h, xs8l).

            The x8 slices go FIRST: the weave projection of batch b starts
            consuming them within ~2us of emission, while the staggered
            attention-phase singles (bt8[h], wp) are not needed until head h
            of the CURRENT batch's attention (b==0), several us later.
            """
            x8h = x8_p.tile([128, 2, N], f8e4, tag="x8h", name=f"x8h{b}")
            x8l = x8_p.tile([128, 2, N], f8e4, tag="x8l", name=f"x8l{b}")
            for ns in range(3):
                n0 = ns * 512
                nsz = min(512, N - n0)
                for nm, t in (("x8h", x8h), ("x8l", x8l)):
                    nc.sync.dma_start(
                        t[:, :, n0:n0 + nsz],
                        a[nm][b, :, :, n0:n0 + nsz].rearrange("c p n -> p c n"),
                    )
                if first and ns == 1:
                    nc.sync.dma_start(wq8h, a["wq8h"].rearrange("c p j -> p c j"))
                    nc.sync.dma_start(wq8l, a["wq8l"].rearrange("c p j -> p c j"))
                    nc.sync.dma_start(bqs, a["bq"])
            xs8h = xs8_p.tile([128, 2, NQ], f8e4, tag="xs8h", name=f"xs8h{b}")
            nc.sync.dma_start(xs8h, a["xs8h"][b].rearrange("c p n -> p c n"))
            xs8l = xs8_p.tile([128, 2, NQ], f8e4, tag="xs8l", name=f"xs8l{b}")
            nc.sync.dma_start(xs8l, a["xs8l"][b].rearrange("c p n -> p c n"))
            if stagger:
                for hh in range(2, 8):
                    nc.sync.dma_start(bt8s[hh], a["bt8"][hh])
                nc.sync.dma_start(wp, a["wpt"].rearrange("c p j -> p c j"))
                nc.sync.dma_start(bps, a["bps"])
            return x8h, x8l, xs8h, xs8l

        def proj_gen(b, x8h, x8l, xs8h, xs8l):
            """Yield after each proj psum tile; returns (kt, vt, qt) eagerly."""
            kt = kt_p.tile([128, 4, N], f32r, tag="kt", name=f"kt{b}")
            vt = v_p.tile([128, NCH, 1024], bf16, tag="vt", name=f"vt{b}")
            qt = qt_p.tile([128, 4, NQ], f32r, tag="qt", name=f"qt{b}")
            kterms = ((wk8h, x8h), (wk8h, x8l), (wk8l, x8h))

            def emit():
                # kT projection: fp8 hi/lo 3-term DoubleRow, n-major so the
                # first psum tile only needs DMA slice 0. No bias
                # (softmax-invariant). Evac via 2D-strided copies.
                for ns in range(2):             # n slices 0:512, 512:1024
                    n0 = ns * 512
                    for prh in range(2):        # pr pairs (0,1), (2,3)
                        ps = sg_tile()
                        for j in range(2):
                            pr = 2 * prh + j
                            for pi, (ww, xx) in enumerate(kterms):
                                nc.tensor.matmul(
                                    ps[:, j, :],
                                    lhsT=ww[:, :, pr * 128:(pr + 1) * 128],
                                    rhs=xx[:, :, n0:n0 + 512],
                                    start=(pi == 0), stop=(pi == 2),
                                    perf_mode=PM.DoubleRow,
                                )
                        if (ns + prh) % 2 == 0:
                            nc.vector.tensor_copy(
                                kt[:, 2 * prh:2 * prh + 2, n0:n0 + 512], ps)
                        else:
                            nc.scalar.copy(
                                kt[:, 2 * prh:2 * prh + 2, n0:n0 + 512], ps)
                        yield
                ps = sg_tile()                  # 256-col tails, two prs per tile
                for prh in range(2):
                    for j in range(2):
                        pr = 2 * prh + j
                        c0 = j * 256
                        for pi, (ww, xx) in enumerate(kterms):
                            nc.tensor.matmul(
                                ps[:, prh, c0:c0 + 256],
                                lhsT=ww[:, :, pr * 128:(pr + 1) * 128],
                                rhs=xx[:, :, 1024:N],
                                start=(pi == 0), stop=(pi == 2),
                                perf_mode=PM.DoubleRow,
                            )
                    nc.scalar.copy(
                        kt[:, 2 * prh:2 * prh + 2, 1024:N],
                        ps[:, prh, :].rearrange("p (g c) -> p g c", g=2))
                yield
                # q projection: fp8 3-term, 2 prs per tile, fused bias on DVE
                qterms = ((wq8h, xs8h), (wq8h, xs8l), (wq8l, xs8h))
                for half in range(2):
                    ps = sg_tile()
                    for j in range(2):
                        pr = 2 * half + j
                        for pi, (ww, xx) in enumerate(qterms):
                            nc.tensor.matmul(
                                ps[:, j, :NQ],
                                lhsT=ww[:, :, pr * 128:(pr + 1) * 128],
                                rhs=xx,
                                start=(pi == 0), stop=(pi == 2),
                                perf_mode=PM.DoubleRow,
                            )
                    nc.vector.tensor_tensor(
                        qt[:, 2 * half:2 * half + 2, :], ps[:, :, :NQ],
                        bqs[:, 2 * half:2 * half + 2].to_broadcast((128, 2, NQ)),
                        ALU.add,
                    )
                    yield
                # v projection: fp8 hi/lo DoubleRow (K=256 per pass, 3 passes)
                for cn in range(NCH):
                    ps = sg_tile()
                    for hf in range(2):
                        for pi, (xx, ww) in enumerate(
                                ((x8h, wv8h), (x8l, wv8h), (x8h, wv8l))):
                            nc.tensor.matmul(
                                ps[:, hf, :],
                                lhsT=xx[:, :, cn * 128:(cn + 1) * 128],
                                rhs=ww[:, :, hf * 512:(hf + 1) * 512],
                                start=(pi == 0), stop=(pi == 2),
                                perf_mode=PM.DoubleRow,
                            )
                    if cn % 2 == 0:
                        nc.vector.tensor_copy(
                            vt[:, cn, :], ps.rearrange("p g j -> p (g j)"))
                    else:
                        nc.scalar.copy(
                            vt[:, cn, :], ps.rearrange("p g j -> p (g j)"))
                    yield

            return kt, vt, qt, emit()

        def attention(b, h, kt, vt, qt, t2):
            # generator: yields after each score group so the driver can
            # weave projection tiles of the next batch between groups
            yield
            pr, p0 = h // 2, 64 * (h % 2)
            bt8 = bt8s[h]
            po = ps_o.tile([128, NQ], f32, tag="po", name=f"po_{b}_{h}")
            acc = None
            e0 = None
            for g in range(NCH // GRP):
                sg = sg_tile()
                for j in range(GRP):
                    c = GRP * g + j
                    nc.tensor.matmul(
                        sg[:, j, :NQ],
                        lhsT=kt[p0:p0 + 64, pr, c * 128:(c + 1) * 128],
                        rhs=qt[p0:p0 + 64, pr, :],
                        start=True, stop=False,
                    )
                    nc.tensor.matmul(
                        sg[:, j, :NQ],
                        lhsT=identp[:, j, :, :],
                        rhs=bt8[:, GRP * g:GRP * (g + 1), :],
                        start=False, stop=True,
                        perf_mode=PM.DoubleRow,
                    )
                e = e_p.tile([128, GRP, NQ], bf16)
                nc.scalar.activation(e, sg[:, :, :NQ], AF.Exp)
                # incremental e-sum accumulation: spreads the adds through
                # the head so the denominator is ready right after the last
                # exp (no end-of-head serial DVE burst blocking the PE SEQ)
                if g == 0:
                    e0 = e
                elif g == 1:
                    acc = esum_p.tile([128, GRP, NQ], bf16, tag="acc")
                    nc.vector.tensor_tensor(acc, e0, e, ALU.add)
                else:
                    nc.vector.tensor_tensor(acc, acc, e, ALU.add)
                    if g == NCH // GRP - 1:
                        esum = esum_p.tile([128, NQ], bf16, tag="esum")
                        nc.vector.tensor_tensor(
                            esum, acc[:, 0, :], acc[:, 1, :], ALU.add)
                for j in range(GRP):
                    c = GRP * g + j
                    nc.tensor.matmul(
                        po[:, :NQ],
                        lhsT=vt[:, c, h * 128:(h + 1) * 128],
                        rhs=e[:, j, :],
                        start=(c == 0), stop=(c == NCH - 1),
                    )
                yield
            # softmax denominator: the fold ran right after the last in-head
            # add, so this matmul's input is ready with no serial DVE burst
            psm = ps_sum.tile([128, NQ], f32, tag="psm", name=f"psm_{b}_{h}")
            nc.tensor.matmul(psm, lhsT=ones, rhs=esum, start=True, stop=True)
            # copy po out of PSUM immediately (decouples the po bank from the
            # recip chain), then normalize on Pool
            oraw = oh_p.tile([128, NQ], bf16, tag="oraw", name=f"oraw_{b}_{h}")
            nc.vector.tensor_copy(oraw, po)
            rc = rc_p.tile([128, NQ], bf16)
            nc.vector.reciprocal(rc, psm)
            oh = oh_p.tile([128, NQ], bf16)
            nc.gpsimd.tensor_tensor(oh, oraw, rc, ALU.mult)
            # hswish: t = min(Relu(o + 3 + bv), 6);  th = (o + bv) * t
            t1 = t1_p.tile([128, NQ], bf16)
            nc.gpsimd.tensor_scalar(t1, oh, bvs3[:, h:h + 1], 0.0, ALU.add, ALU.max)
            nc.gpsimd.tensor_scalar(t1, t1, 6.0, None, ALU.min)
            nc.gpsimd.scalar_tensor_tensor(
                t2[:, h, b % 2, :], oh, bvs[:, h:h + 1], t1, ALU.add, ALU.mult)

        def out_proj(b, t2, qcs=range(5)):
            for qc in qcs:
                r0 = (b - 1) * NQ + qc * 128
                ps = sg_tile()
                for dc in range(8):
                    nc.tensor.matmul(
                        ps[:, 0, :OUT],
                        lhsT=t2[:, dc, :, :].rearrange(
                            "p bb q -> p (bb q)")[:, qc * 128:(qc + 1) * 128],
                        rhs=wp[:, dc, :],
                        start=(dc == 0), stop=(dc == 7),
                    )
                ob = ob_p.tile([128, OUT], f32)
                nc.vector.tensor_tensor(ob, ps[:, 0, :OUT], bps, ALU.add)
                nc.sync.dma_start(out_flat[r0:r0 + 128, :], ob)
                yield

        # prologue: batch 0 inputs + weights; proj(0) up to attention-ready
        x8h0, x8l0, xs8h0, xs8l0 = dma_x(0, first=True)
        nc.sync.dma_start(identp, a["identp"])
        nc.sync.dma_start(wv8h, a["wv8h"].rearrange("c p j -> p c j"))
        nc.sync.dma_start(bt8s[0], a["bt8"][0])
        nc.sync.dma_start(wv8l, a["wv8l"].rearrange("c p j -> p c j"))
        nc.sync.dma_start(bt8s[1], a["bt8"][1])
        nc.sync.dma_start(ones, a["ones"])
        nc.sync.dma_start(bvs, a["bv"])
        nc.sync.dma_start(bvs3, a["bv3"])

        # warm up the PE p-state during the input-DMA wait: dummy matmuls on
        # a memset scratch (no DMA dependency) bridge the ~3.2us DMA pipe
        # latency continuously, so the first real matmul runs at full clock
        scratch = singles.tile([128, 2, 256], f8e4)
        nc.gpsimd.memset(scratch, 0.0)
        warm = sg_tile()
        for i in range(14):
            nc.tensor.matmul(
                warm[:, 0, :256],
                lhsT=scratch[:, :, 0:128],
                rhs=scratch,
                start=True, stop=True,
                perf_mode=PM.DoubleRow,
            )

        kt, vt, qt, gen0 = proj_gen(0, x8h0, x8l0, xs8h0, xs8l0)
        for _ in gen0:          # batch 0 proj must fully precede its attention
            pass
        pending = []
        t2 = None
        nxt = None
        for b in range(BPC):
            if b % 2 == 0:
                t2 = hs_p.tile([128, H, 2, NQ], bf16, tag="t2", name=f"t2_{b}")
            # interleave remaining proj tiles (this batch's tail + next batch)
            if b + 1 < BPC:
                x8hn, x8ln, xs8hn, xs8ln = dma_x(b + 1, stagger=(b == 0))
                nxt = proj_gen(b + 1, x8hn, x8ln, xs8hn, xs8ln)
                pending.append(nxt[3])

            for h in range(H):
                for gi, _ in enumerate(attention(b, h, kt, vt, qt, t2)):
                    for _ in range(3 if gi == 5 else (1 if gi else 0)):
                        while pending:
                            if next(pending[0], "done") == "done":
                                pending.pop(0)
                            else:
                                break
                if b == BPC - 1 and h == 4:
                    # tail trim: out-proj columns 0,1 of the last pair only
                    # read the PRIOR batch's t2 half -- weave them into the
                    # last heads of the final batch
                    pending.append(out_proj(b, t2, qcs=range(2)))
            while pending:
                if next(pending[0], "done") == "done":
                    pending.pop(0)
                else:
                    break
            if pending:
                for _ in pending[0]:
                    pass
                pending.pop(0)
            if b % 2 == 1:
                if b + 1 < BPC:
                    pending.append(out_proj(b, t2))  # interleave with next batch
                else:
                    for _ in out_proj(b, t2, qcs=range(2, 5)):
                        pass
            if nxt is not None:
                kt, vt, qt = nxt[0], nxt[1], nxt[2]
                nxt = None


def build():
    import concourse.mybir as mybir
    import concourse.tile as tile
    from concourse import bacc

    nc = bacc.Bacc("TRN2", target_bir_lowering=False, debug=False)
    f32, bf16 = mybir.dt.float32, mybir.dt.bfloat16
    f8e4 = mybir.dt.float8e4
    a = {}

    def din(name, shape, dt=f32):
        a[name] = nc.dram_tensor(name, shape, dt, kind="ExternalInput").ap()

    din("x8h", [BPC, 2, 128, N], f8e4)
    din("x8l", [BPC, 2, 128, N], f8e4)
    din("xs8h", [BPC, 2, 128, NQ], f8e4)
    din("xs8l", [BPC, 2, 128, NQ], f8e4)
    din("wk8h", [2, 128, 512], f8e4)
    din("wk8l", [2, 128, 512], f8e4)
    din("wq8h", [2, 128, 512], f8e4)
    din("wq8l", [2, 128, 512], f8e4)
    din("wv8h", [2, 128, 1024], f8e4)
    din("wv8l", [2, 128, 1024], f8e4)
    din("wpt", [8, 128, OUT], bf16)
    din("bq", [128, 4])
    din("bv", [128, H])
    din("bv3", [128, H])
    din("bps", [128, OUT])
    din("bt8", [H, 128, NCH, NQ], f8e4)
    din("ones", [128, 128], bf16)
    din("identp", [128, 2, 2, 128], f8e4)
    out_ap = nc.dram_tensor("out", [BPC, NQ, OUT], f32, kind="ExternalOutput").ap()

    with tile.TileContext(nc) as tc:
        _body(tc, a, out_ap)
    nc.compile()
    return nc


_NC_CACHE = None


def _get_nc():
    global _NC_CACHE
    if _NC_CACHE is None:
        _NC_CACHE = build()
    return _NC_CACHE


def kernel(**inputs):
    from concourse.bass_utils import run_bass_kernel_spmd

    in_maps = _prep(inputs)
    nc = _get_nc()
    res = run_bass_kernel_spmd(nc, in_maps, list(range(NCORES)))
    out = np.concatenate([res.results[i]["out"] for i in range(NCORES)], axis=0)
    return np.ascontiguousarray(out, dtype=np.float32)


if __name__ == "__main__":
    rng = np.random.default_rng(0)
    print("smoke: building bass module...")
    nc = build()
    print("built ok:", sum(len(bb.instructions) for bb in nc.m.functions[0].blocks), "instructions")

